# revision 2
# baseline (speedup 1.0000x reference)
"""Trainium2 Bass kernel v2: sequential relu-chain class-max.

Pipeline: ConvTranspose3d(16->32, k=3, s=2, p=1, op=1) -> MaxPool3d(2)
          -> softmax(ch) -> subtract -> swish -> max(ch)

v2 key idea (vs v1's relu-pairmax + DVE j-reduce):
  max(a1..a8) computed as a sequential relu chain IN PSUM:
      P = a8-a7; relu; P += a7-a6; relu; ...; P += a1
  so the whole 8-class pool-max costs 8 matmul rounds (same PE total as
  v1) plus 7 relu passes of only 32 values/pos each (224/pos), versus
  v1's relu(128/pos) + DVE tensor_reduce(128/pos) = 256/pos.  Crucially
  every pass is relu-shaped, so it SPLITS between Act and DVE, killing
  v1's irreducible 151us DVE reduce wall.  Conv bias lands via one K=1
  matmul per plane on the finished chain cell (v1 burned 55us of PE on
  per-group bias matmuls).

Sharding: data-parallel over batch B=16 -> 2 per core x 8 cores.
"""

import sys

sys.path.insert(0, "/opt/trn_rl_repo")

import numpy as np
import ml_dtypes

# ---------------------------------------------------------------- constants
IN_C, OUT_C, K, STRIDE, PAD, OUT_PAD = 16, 32, 3, 2, 1, 1
B, D, H, W = 16, 16, 64, 64
N_CORES = 8
B_PER_CORE = B // N_CORES  # 2

PLANE = H * W            # 4096 positions per (b, d) plane
BLK = 128                # positions per matmul block
BLKS = PLANE // BLK      # 32
NSTEP = 8                # chain matmul rounds (7 diffs + final)

X_NP_DT = ml_dtypes.bfloat16
W_NP_DT = ml_dtypes.bfloat16

_COMPILED = {}


def _tap(o, p):
    """Kernel tap index used by parity class p at window offset o, or None."""
    if p == 0:
        return 1 if o == 0 else None
    return 2 if o == 0 else 0


def build_w8(weight):
    """[128 rows=(od,oh,ow,cin), 2,2,2,32 cols=(pd,ph,pw,c)] conv matrix."""
    wr = np.zeros((2, 2, 2, IN_C, 2, 2, 2, OUT_C), dtype=np.float32)
    for od in range(2):
        for oh in range(2):
            for ow in range(2):
                for pd in range(2):
                    kd = _tap(od, pd)
                    if kd is None:
                        continue
                    for ph in range(2):
                        kh = _tap(oh, ph)
                        if kh is None:
                            continue
                        for pw in range(2):
                            kw = _tap(ow, pw)
                            if kw is None:
                                continue
                            wr[od, oh, ow, :, pd, ph, pw, :] = weight[:, :, kd, kh, kw]
    return wr.reshape(128, 8, OUT_C)  # [K, class, c]


def build_wchain(weight):
    """[128 K, NSTEP, 32] chain weights: steps 0..6 = W[s+1]-W[s] reversed,
    step 7 = W[cls0] exact.  Chain: P=W7-W6; relu; +=W6-W5; ...; +=W0."""
    w8 = build_w8(weight)  # [128, 8, 32]
    ch = np.empty((128, NSTEP, OUT_C), dtype=np.float32)
    for s in range(7):
        ch[:, s] = w8[:, 7 - s] - w8[:, 6 - s]
    ch[:, 7] = w8[:, 0]
    return ch


def build_xstack(x):
    """[B, D, 128 rows=(od,oh,ow,cin), PLANE] shifted/padded copies of x."""
    xp = np.zeros((B, IN_C, D + 1, H + 1, W + 1), dtype=np.float32)
    xp[:, :, :D, :H, :W] = x
    S = np.empty((B, D, 2, 2, 2, IN_C, H, W), dtype=X_NP_DT)
    for od in range(2):
        for oh in range(2):
            for ow in range(2):
                sl = xp[:, :, od:od + D, oh:oh + H, ow:ow + W]
                S[:, :, od, oh, ow] = sl.transpose(0, 2, 1, 3, 4).astype(X_NP_DT)
    return S.reshape(B, D, 128, PLANE)


def build_kernel(relu_acts=(0, 2, 4, 6)):
    from concourse import bass, bacc, mybir, tile

    f32 = mybir.dt.float32
    bf16 = mybir.dt.bfloat16
    Alu = mybir.AluOpType
    Act = mybir.ActivationFunctionType
    Ax = mybir.AxisListType

    nc = bacc.Bacc("TRN2", target_bir_lowering=False, debug=False,
                   num_devices=N_CORES)

    xs_h = nc.declare_dram_parameter("xs", [B_PER_CORE, D, 128, PLANE],
                                     bf16, isOutput=False)
    wc_h = nc.declare_dram_parameter("wc", [128, NSTEP * OUT_C], bf16,
                                     isOutput=False)
    # brow: [1, 128+1024]: cols 0:128 ones, 128:1152 bias pattern (blk, c)
    br_h = nc.declare_dram_parameter("brow", [1, 128 + BLKS * OUT_C], bf16,
                                     isOutput=False)
    sub_h = nc.declare_dram_parameter("subrep", [128, BLKS * OUT_C], bf16,
                                      isOutput=False)
    id_h = nc.declare_dram_parameter("ident", [128, 128], f32, isOutput=False)
    y_h = nc.declare_dram_parameter("y", [B_PER_CORE, D, PLANE], f32,
                                    isOutput=True)

    with tile.TileContext(nc) as tc:
        with (
            tc.tile_pool(name="const", bufs=1) as constp,
            tc.tile_pool(name="xslab", bufs=6) as xpool,
            tc.tile_pool(name="cell", bufs=4, space="PSUM") as cellp,
            tc.tile_pool(name="E", bufs=2) as ep,
            tc.tile_pool(name="z", bufs=2) as zp,
            tc.tile_pool(name="sm", bufs=2) as smp,
            tc.tile_pool(name="vv", bufs=2) as vp,
            tc.tile_pool(name="mm", bufs=2) as mmp,
            tc.tile_pool(name="ext", bufs=2) as extp,
            tc.tile_pool(name="sil", bufs=2) as silp,
            tc.tile_pool(name="ost", bufs=2) as ostp,
            tc.tile_pool(name="tout", bufs=2) as toutp,
        ):
            wc = constp.tile([128, NSTEP, OUT_C], bf16, name="wc")
            nc.sync.dma_start(wc[:].rearrange("p s c -> p (s c)"), wc_h[:, :])
            brow = constp.tile([1, 128 + BLKS * OUT_C], bf16, name="brow")
            nc.sync.dma_start(brow[:], br_h[:, :])
            subrep = constp.tile([128, BLKS, OUT_C], bf16, name="subrep")
            ident = constp.tile([128, 128], f32, name="ident")
            siluwarm = constp.tile([1, 2], f32, name="siluwarm")
            nc.scalar.activation(siluwarm[:, 0:1], brow[0:1, 0:1], Act.Silu)
            nc.scalar.activation(siluwarm[:, 1:2], brow[0:1, 0:1], Act.Exp)
            deferred = [False]

            def emit_deferred_consts():
                nc.sync.dma_start(
                    subrep[:].rearrange("p a c -> p (a c)"), sub_h[:, :])
                nc.sync.dma_start(ident[:], id_h[:, :])
                deferred[0] = True

            def emit_chain_step(slab, cell, s):
                """one matmul round + its relu (relu split Act/DVE)."""
                for b in range(BLKS):
                    nc.tensor.matmul(
                        cell[:, b, :], slab[:, b * BLK:(b + 1) * BLK],
                        wc[:, s, :], start=(s == 0), stop=True,
                        skip_group_check=(s != 0))
                if s < NSTEP - 1:
                    flat = cell[:].rearrange("p a c -> p (a c)")
                    if s in relu_acts:
                        nc.scalar.activation(flat, flat, Act.Relu)
                    else:
                        nc.vector.tensor_scalar_max(flat, flat, 0.0)

            def emit_bias(cell):
                # one K=1 matmul per psum bank (a matmul output cannot span
                # banks): adds the conv bias onto the finished chain cell
                for h in range(2):
                    nc.tensor.matmul(
                        cell[:, 16 * h:16 * (h + 1), :].rearrange(
                            "p a c -> p (a c)"),
                        brow[0:1, 0:128], brow[0:1, 128 + 512 * h:640 + 512 * h],
                        start=False, stop=True, skip_group_check=True)

            def emit_exp(cell):
                """exp straight out of the finished chain cell (frees psum)."""
                E = ep.tile([128, BLKS, OUT_C], f32, tag="E", name="E")
                nc.scalar.activation(
                    E[:].rearrange("p a c -> p (a c)"),
                    cell[:].rearrange("p a c -> p (a c)"), Act.Exp)
                return E

            def emit_tail_a(E):
                """softmax denominator for a finished plane."""
                # Z-tree on Pool: 32 -> 16 -> 8 -> 4 -> 2 -> 1
                e1 = mmp.tile([128, BLKS, 16], f32, tag="e1", name="e1")
                nc.gpsimd.tensor_tensor(e1[:], E[:, :, 0:16], E[:, :, 16:32],
                                        Alu.add)
                e2 = mmp.tile([128, BLKS, 8], f32, tag="e2", name="e2")
                nc.gpsimd.tensor_tensor(e2[:], e1[:, :, 0:8], e1[:, :, 8:16],
                                        Alu.add)
                e3 = mmp.tile([128, BLKS, 4], f32, tag="e3", name="e3")
                nc.gpsimd.tensor_tensor(e3[:], e2[:, :, 0:4], e2[:, :, 4:8],
                                        Alu.add)
                e4 = mmp.tile([128, BLKS, 2], f32, tag="e4", name="e4")
                nc.gpsimd.tensor_tensor(e4[:], e3[:, :, 0:2], e3[:, :, 2:4],
                                        Alu.add)
                Z = zp.tile([128, BLKS], f32, tag="Z", name="Z")
                nc.gpsimd.tensor_tensor(Z[:], e4[:, :, 0], e4[:, :, 1],
                                        Alu.add)
                R = zp.tile([128, BLKS], f32, tag="R", name="R")
                nc.vector.reciprocal(R[:], Z[:])
                return R

            def emit_b_final(b, ext, half=None):
                """silu on the per-b extremes, final max, output DMA."""
                lo, n = (0, 512) if half is None else (half * 256, 256)
                sil = silp.tile([128, 2, n], f32, tag=f"sil{n}", name="sil")
                nc.scalar.activation(sil[:], ext[:, :, lo:lo + n], Act.Silu)
                ost = ostp.tile([128, n], f32, tag=f"ost{n}", name="ost")
                nc.vector.tensor_tensor(ost[:], sil[:, 0, :], sil[:, 1, :],
                                        Alu.max)
                nj = n // 128
                tpc = cellp.tile([128, BLKS, OUT_C], f32, tag="cell",
                                 name="tpc")
                tp = tpc[:].rearrange("p a c -> p (a c)")[
                    :, 0:nj * 128].rearrange("p (a c) -> p a c", a=nj, c=128)
                for j in range(nj):
                    nc.tensor.transpose(tp[:, j, :],
                                        ost[:, 128 * j:128 * (j + 1)],
                                        ident[:])
                T = toutp.tile([128, nj, 128], f32, tag=f"T{n}", name="T")
                nc.scalar.activation(
                    T[:].rearrange("p a c -> p (a c)"),
                    tp[:].rearrange("p a c -> p (a c)"), Act.Copy)
                nc.sync.dma_start(
                    y_h[b].flatten().rearrange(
                        "(j r p) -> r j p", j=4, r=BLK,
                        p=BLK)[:, lo // 128:lo // 128 + nj, :],
                    T[:])

            exts = []
            for b in range(B_PER_CORE):
                ext_b = extp.tile([128, 2, D * BLKS], f32,
                                  tag=f"ext{b}", name=f"ext{b}")
                exts.append(ext_b)

            NP = B_PER_CORE * D  # 32 planes
            slabs, cells, plane_info = {}, {}, {}
            tails = {}

            def emit_dma(k):
                if k >= NP:
                    return
                b, d = divmod(k, D)
                slab = xpool.tile([128, PLANE], bf16, tag="slab", name="slab")
                for g in range(4):
                    c0 = g * 1024
                    nc.sync.dma_start(slab[:, c0:c0 + 1024],
                                      xs_h[b, d, :, c0:c0 + 1024])
                slabs[k] = slab
                plane_info[k] = (b, d)

            def start_plane(k):
                cell = cellp.tile([128, BLKS, OUT_C], f32, tag="cell",
                                  name="cell")
                cells[k] = cell

            def pend_of(k):
                b, d = plane_info[k]
                return (None, exts[b], d * BLKS, d, b)

            Es = {}
            VS = {}

            def emit_tail_d1(k):
                """Z-tree, recip, sm, v for plane k (E already computed)."""
                E = Es.pop(k)
                R = emit_tail_a(E)
                sm = smp.tile([128, BLKS, OUT_C], bf16, tag="sm", name="sm")
                nc.gpsimd.tensor_tensor(
                    sm[:], E[:],
                    R[:].unsqueeze(2).broadcast_to([128, BLKS, OUT_C]),
                    Alu.mult)
                v = vp.tile([128, BLKS, OUT_C], bf16, tag="v", name="v")
                nc.gpsimd.tensor_tensor(v[:], sm[:], subrep[:], Alu.subtract)
                VS[k] = v

            def emit_tail_d2(k):
                """channel max/min extremes for plane k."""
                v = VS.pop(k)
                _, ext, col, _, _ = pend_of(k)
                m1 = mmp.tile([128, BLKS, 16], bf16, tag="m1", name="m1")
                nc.vector.tensor_tensor(m1[:], v[:, :, 0:16], v[:, :, 16:32],
                                        Alu.max)
                m2 = mmp.tile([128, BLKS, 8], bf16, tag="m2", name="m2")
                nc.vector.tensor_tensor(m2[:], m1[:, :, 0:8], m1[:, :, 8:16],
                                        Alu.max)
                nc.vector.tensor_reduce(
                    ext[:, 0, col:col + BLKS], m2[:], axis=Ax.X, op=Alu.max)
                s1 = mmp.tile([128, BLKS, 16], f32, tag="s1", name="s1")
                nc.gpsimd.tensor_tensor(s1[:], v[:, :, 0:16], v[:, :, 16:32],
                                        Alu.add)
                n1 = mmp.tile([128, BLKS, 16], bf16, tag="n1", name="n1")
                nc.gpsimd.tensor_tensor(n1[:], s1[:], m1[:], Alu.subtract)
                n2 = mmp.tile([128, BLKS, 8], bf16, tag="n2", name="n2")
                nc.vector.tensor_tensor(n2[:], n1[:, :, 0:8], n1[:, :, 8:16],
                                        Alu.min)
                nc.vector.tensor_reduce(
                    ext[:, 1, col:col + BLKS], n2[:], axis=Ax.X, op=Alu.min)

            # Four-deep skewed pipeline: per iteration t, plane t runs steps
            # 0-1, t-1 runs 2-3, t-2 runs 4-5, t-3 runs 6-7+bias+exp (which
            # frees its psum cell); plane t-4 gets Z/recip/sm/v and t-5 its
            # chmax, each emitted in dependency-readiness order so no engine
            # queue head waits on same-iteration work.  4 cells live = all 8
            # psum banks; output transposes borrow a freed cell buffer.
            emit_dma(0)
            emit_dma(1)
            for t in range(NP + 6):
                if 0 <= t - 5 < NP:
                    emit_tail_d2(t - 5)

                if 0 <= t - 3 < NP:
                    k = t - 3
                    emit_chain_step(slabs[k], cells[k], 6)
                    emit_chain_step(slabs[k], cells[k], 7)
                    emit_bias(cells[k])
                    Es[k] = emit_exp(cells[k])
                    del cells[k]
                    del slabs[k]
                if 0 <= t - 4 < NP:
                    emit_tail_d1(t - 4)
                if 0 <= t - 2 < NP:
                    emit_chain_step(slabs[t - 2], cells[t - 2], 4)
                    emit_chain_step(slabs[t - 2], cells[t - 2], 5)
                if 0 <= t - 1 < NP:
                    emit_chain_step(slabs[t - 1], cells[t - 1], 2)
                    emit_chain_step(slabs[t - 1], cells[t - 1], 3)
                if t < NP:
                    start_plane(t)
                    emit_dma(t + 2)
                    if not deferred[0]:
                        emit_deferred_consts()
                    emit_chain_step(slabs[t], cells[t], 0)
                    emit_chain_step(slabs[t], cells[t], 1)
            emit_b_final(0, exts[0])
            emit_b_final(1, exts[1])

    nc.compile()
    return nc


def _get_nc():
    if "nc" not in _COMPILED:
        _COMPILED["nc"] = build_kernel()
    return _COMPILED["nc"]


def build_in_maps(xs, wch, bias, subtract):
    brow = np.concatenate([
        np.ones(128, np.float32),
        np.tile(bias.astype(np.float32), BLKS),
    ]).reshape(1, 128 + BLKS * OUT_C).astype(ml_dtypes.bfloat16)
    subrep = np.tile(subtract[None, None, :], (128, BLKS, 1)).reshape(
        128, BLKS * OUT_C).astype(ml_dtypes.bfloat16)

    in_maps = []
    for c in range(N_CORES):
        in_maps.append({
            "xs": np.ascontiguousarray(xs[c * B_PER_CORE:(c + 1) * B_PER_CORE]),
            "wc": wch,
            "brow": brow,
            "subrep": subrep,
            "ident": np.eye(128, dtype=np.float32),
        })
    return in_maps


def kernel(x, weight, bias, subtract):
    from concourse.bass_utils import run_bass_kernel_spmd

    x = np.asarray(x, dtype=np.float32)
    weight = np.asarray(weight, dtype=np.float32)
    bias = np.asarray(bias, dtype=np.float32)
    subtract = np.asarray(subtract, dtype=np.float32)

    nc = _get_nc()

    xs = build_xstack(x)
    wch = build_wchain(weight).reshape(128, NSTEP * OUT_C).astype(W_NP_DT)
    in_maps = build_in_maps(xs, wch, bias, subtract)

    res = run_bass_kernel_spmd(nc, in_maps, core_ids=list(range(N_CORES)))
    outs = [res.results[c]["y"].reshape(B_PER_CORE, D, H, W)
            for c in range(N_CORES)]
    return np.concatenate(outs, axis=0)


# revision 3
# speedup vs baseline: 1.0028x; 1.0028x over previous
"""Trainium2 Bass kernel v2: sequential relu-chain class-max.

Pipeline: ConvTranspose3d(16->32, k=3, s=2, p=1, op=1) -> MaxPool3d(2)
          -> softmax(ch) -> subtract -> swish -> max(ch)

v2 key idea (vs v1's relu-pairmax + DVE j-reduce):
  max(a1..a8) computed as a sequential relu chain IN PSUM:
      P = a8-a7; relu; P += a7-a6; relu; ...; P += a1
  so the whole 8-class pool-max costs 8 matmul rounds (same PE total as
  v1) plus 7 relu passes of only 32 values/pos each (224/pos), versus
  v1's relu(128/pos) + DVE tensor_reduce(128/pos) = 256/pos.  Crucially
  every pass is relu-shaped, so it SPLITS between Act and DVE, killing
  v1's irreducible 151us DVE reduce wall.  Conv bias lands via one K=1
  matmul per plane on the finished chain cell (v1 burned 55us of PE on
  per-group bias matmuls).

Sharding: data-parallel over batch B=16 -> 2 per core x 8 cores.
"""

import sys

sys.path.insert(0, "/opt/trn_rl_repo")

import numpy as np
import ml_dtypes

# ---------------------------------------------------------------- constants
IN_C, OUT_C, K, STRIDE, PAD, OUT_PAD = 16, 32, 3, 2, 1, 1
B, D, H, W = 16, 16, 64, 64
N_CORES = 8
B_PER_CORE = B // N_CORES  # 2

PLANE = H * W            # 4096 positions per (b, d) plane
BLK = 128                # positions per matmul block
BLKS = PLANE // BLK      # 32
NSTEP = 8                # chain matmul rounds (7 diffs + final)

X_NP_DT = ml_dtypes.bfloat16
W_NP_DT = ml_dtypes.bfloat16

_COMPILED = {}


def _tap(o, p):
    """Kernel tap index used by parity class p at window offset o, or None."""
    if p == 0:
        return 1 if o == 0 else None
    return 2 if o == 0 else 0


def build_w8(weight):
    """[128 rows=(od,oh,ow,cin), 2,2,2,32 cols=(pd,ph,pw,c)] conv matrix."""
    wr = np.zeros((2, 2, 2, IN_C, 2, 2, 2, OUT_C), dtype=np.float32)
    for od in range(2):
        for oh in range(2):
            for ow in range(2):
                for pd in range(2):
                    kd = _tap(od, pd)
                    if kd is None:
                        continue
                    for ph in range(2):
                        kh = _tap(oh, ph)
                        if kh is None:
                            continue
                        for pw in range(2):
                            kw = _tap(ow, pw)
                            if kw is None:
                                continue
                            wr[od, oh, ow, :, pd, ph, pw, :] = weight[:, :, kd, kh, kw]
    return wr.reshape(128, 8, OUT_C)  # [K, class, c]


def build_wchain(weight):
    """[128 K, NSTEP, 32] chain weights: steps 0..6 = W[s+1]-W[s] reversed,
    step 7 = W[cls0] exact.  Chain: P=W7-W6; relu; +=W6-W5; ...; +=W0."""
    w8 = build_w8(weight)  # [128, 8, 32]
    ch = np.empty((128, NSTEP, OUT_C), dtype=np.float32)
    for s in range(7):
        ch[:, s] = w8[:, 7 - s] - w8[:, 6 - s]
    ch[:, 7] = w8[:, 0]
    return ch


def build_xstack(x):
    """[B, D, 128 rows=(od,oh,ow,cin), PLANE] shifted/padded copies of x."""
    xp = np.zeros((B, IN_C, D + 1, H + 1, W + 1), dtype=np.float32)
    xp[:, :, :D, :H, :W] = x
    S = np.empty((B, D, 2, 2, 2, IN_C, H, W), dtype=X_NP_DT)
    for od in range(2):
        for oh in range(2):
            for ow in range(2):
                sl = xp[:, :, od:od + D, oh:oh + H, ow:ow + W]
                S[:, :, od, oh, ow] = sl.transpose(0, 2, 1, 3, 4).astype(X_NP_DT)
    return S.reshape(B, D, 128, PLANE)


def build_kernel(relu_acts=(0, 2, 4, 6)):
    from concourse import bass, bacc, mybir, tile

    f32 = mybir.dt.float32
    bf16 = mybir.dt.bfloat16
    Alu = mybir.AluOpType
    Act = mybir.ActivationFunctionType
    Ax = mybir.AxisListType

    nc = bacc.Bacc("TRN2", target_bir_lowering=False, debug=False,
                   num_devices=N_CORES)

    xs_h = nc.declare_dram_parameter("xs", [B_PER_CORE, D, 128, PLANE],
                                     bf16, isOutput=False)
    wc_h = nc.declare_dram_parameter("wc", [128, NSTEP * OUT_C], bf16,
                                     isOutput=False)
    # brow: [1, 128+1024]: cols 0:128 ones, 128:1152 bias pattern (blk, c)
    br_h = nc.declare_dram_parameter("brow", [1, 128 + BLKS * OUT_C], bf16,
                                     isOutput=False)
    sub_h = nc.declare_dram_parameter("subrep", [128, BLKS * OUT_C], bf16,
                                      isOutput=False)
    id_h = nc.declare_dram_parameter("ident", [128, 128], f32, isOutput=False)
    y_h = nc.declare_dram_parameter("y", [B_PER_CORE, D, PLANE], f32,
                                    isOutput=True)

    with tile.TileContext(nc) as tc:
        with (
            tc.tile_pool(name="const", bufs=1) as constp,
            tc.tile_pool(name="xslab", bufs=6) as xpool,
            tc.tile_pool(name="cell", bufs=4, space="PSUM") as cellp,
            tc.tile_pool(name="E", bufs=2) as ep,
            tc.tile_pool(name="z", bufs=2) as zp,
            tc.tile_pool(name="sm", bufs=2) as smp,
            tc.tile_pool(name="vv", bufs=2) as vp,
            tc.tile_pool(name="mm", bufs=2) as mmp,
            tc.tile_pool(name="ext", bufs=2) as extp,
            tc.tile_pool(name="sil", bufs=2) as silp,
            tc.tile_pool(name="ost", bufs=2) as ostp,
            tc.tile_pool(name="tout", bufs=2) as toutp,
        ):
            wc = constp.tile([128, NSTEP, OUT_C], bf16, name="wc")
            nc.sync.dma_start(wc[:].rearrange("p s c -> p (s c)"), wc_h[:, :])
            brow = constp.tile([1, 128 + BLKS * OUT_C], bf16, name="brow")
            nc.sync.dma_start(brow[:], br_h[:, :])
            subrep = constp.tile([128, BLKS, OUT_C], bf16, name="subrep")
            ident = constp.tile([128, 128], f32, name="ident")
            siluwarm = constp.tile([1, 2], f32, name="siluwarm")
            nc.scalar.activation(siluwarm[:, 0:1], brow[0:1, 0:1], Act.Silu)
            nc.scalar.activation(siluwarm[:, 1:2], brow[0:1, 0:1], Act.Exp)
            deferred = [False]

            def emit_deferred_consts():
                nc.sync.dma_start(
                    subrep[:].rearrange("p a c -> p (a c)"), sub_h[:, :])
                nc.sync.dma_start(ident[:], id_h[:, :])
                deferred[0] = True

            def emit_chain_step(slab, cell, s, halved=False):
                """one matmul round + its relu (relu split Act/DVE)."""
                if halved:
                    for h in range(2):
                        for b in range(16 * h, 16 * h + 16):
                            nc.tensor.matmul(
                                cell[:, b, :], slab[:, b * BLK:(b + 1) * BLK],
                                wc[:, s, :], start=(s == 0), stop=True,
                                skip_group_check=(s != 0))
                        if s < NSTEP - 1:
                            flat = cell[:, 16 * h:16 * h + 16, :].rearrange(
                                "p a c -> p (a c)")
                            if h == 0:
                                nc.scalar.activation(flat, flat, Act.Relu)
                            else:
                                nc.vector.tensor_scalar_max(flat, flat, 0.0)
                    return
                for b in range(BLKS):
                    nc.tensor.matmul(
                        cell[:, b, :], slab[:, b * BLK:(b + 1) * BLK],
                        wc[:, s, :], start=(s == 0), stop=True,
                        skip_group_check=(s != 0))
                if s < NSTEP - 1:
                    flat = cell[:].rearrange("p a c -> p (a c)")
                    if s in relu_acts:
                        nc.scalar.activation(flat, flat, Act.Relu)
                    else:
                        nc.vector.tensor_scalar_max(flat, flat, 0.0)

            def emit_bias(cell):
                # one K=1 matmul per psum bank (a matmul output cannot span
                # banks): adds the conv bias onto the finished chain cell
                for h in range(2):
                    nc.tensor.matmul(
                        cell[:, 16 * h:16 * (h + 1), :].rearrange(
                            "p a c -> p (a c)"),
                        brow[0:1, 0:128], brow[0:1, 128 + 512 * h:640 + 512 * h],
                        start=False, stop=True, skip_group_check=True)

            def emit_exp(cell):
                """exp straight out of the finished chain cell (frees psum)."""
                E = ep.tile([128, BLKS, OUT_C], f32, tag="E", name="E")
                nc.scalar.activation(
                    E[:].rearrange("p a c -> p (a c)"),
                    cell[:].rearrange("p a c -> p (a c)"), Act.Exp)
                return E

            def emit_tail_a(E):
                """softmax denominator for a finished plane."""
                # Z-tree on Pool: 32 -> 16 -> 8 -> 4 -> 2 -> 1
                e1 = mmp.tile([128, BLKS, 16], f32, tag="e1", name="e1")
                nc.gpsimd.tensor_tensor(e1[:], E[:, :, 0:16], E[:, :, 16:32],
                                        Alu.add)
                e2 = mmp.tile([128, BLKS, 8], f32, tag="e2", name="e2")
                nc.gpsimd.tensor_tensor(e2[:], e1[:, :, 0:8], e1[:, :, 8:16],
                                        Alu.add)
                e3 = mmp.tile([128, BLKS, 4], f32, tag="e3", name="e3")
                nc.gpsimd.tensor_tensor(e3[:], e2[:, :, 0:4], e2[:, :, 4:8],
                                        Alu.add)
                e4 = mmp.tile([128, BLKS, 2], f32, tag="e4", name="e4")
                nc.gpsimd.tensor_tensor(e4[:], e3[:, :, 0:2], e3[:, :, 2:4],
                                        Alu.add)
                Z = zp.tile([128, BLKS], f32, tag="Z", name="Z")
                nc.gpsimd.tensor_tensor(Z[:], e4[:, :, 0], e4[:, :, 1],
                                        Alu.add)
                R = zp.tile([128, BLKS], f32, tag="R", name="R")
                nc.vector.reciprocal(R[:], Z[:])
                return R

            def emit_b_final(b, ext, half=None):
                """silu on the per-b extremes, final max, output DMA."""
                lo, n = (0, 512) if half is None else (half * 256, 256)
                sil = silp.tile([128, 2, n], f32, tag=f"sil{n}", name="sil")
                nc.scalar.activation(sil[:], ext[:, :, lo:lo + n], Act.Silu)
                ost = ostp.tile([128, n], f32, tag=f"ost{n}", name="ost")
                nc.vector.tensor_tensor(ost[:], sil[:, 0, :], sil[:, 1, :],
                                        Alu.max)
                nj = n // 128
                tpc = cellp.tile([128, BLKS, OUT_C], f32, tag="cell",
                                 name="tpc")
                tp = tpc[:].rearrange("p a c -> p (a c)")[
                    :, 0:nj * 128].rearrange("p (a c) -> p a c", a=nj, c=128)
                for j in range(nj):
                    nc.tensor.transpose(tp[:, j, :],
                                        ost[:, 128 * j:128 * (j + 1)],
                                        ident[:])
                T = toutp.tile([128, nj, 128], f32, tag=f"T{n}", name="T")
                nc.scalar.activation(
                    T[:].rearrange("p a c -> p (a c)"),
                    tp[:].rearrange("p a c -> p (a c)"), Act.Copy)
                nc.sync.dma_start(
                    y_h[b].flatten().rearrange(
                        "(j r p) -> r j p", j=4, r=BLK,
                        p=BLK)[:, lo // 128:lo // 128 + nj, :],
                    T[:])

            exts = []
            for b in range(B_PER_CORE):
                ext_b = extp.tile([128, 2, D * BLKS], f32,
                                  tag=f"ext{b}", name=f"ext{b}")
                exts.append(ext_b)

            NP = B_PER_CORE * D  # 32 planes
            slabs, cells, plane_info = {}, {}, {}
            tails = {}

            def emit_dma(k):
                if k >= NP:
                    return
                b, d = divmod(k, D)
                slab = xpool.tile([128, PLANE], bf16, tag="slab", name="slab")
                for g in range(4):
                    c0 = g * 1024
                    nc.sync.dma_start(slab[:, c0:c0 + 1024],
                                      xs_h[b, d, :, c0:c0 + 1024])
                slabs[k] = slab
                plane_info[k] = (b, d)

            def start_plane(k):
                cell = cellp.tile([128, BLKS, OUT_C], f32, tag="cell",
                                  name="cell")
                cells[k] = cell

            def pend_of(k):
                b, d = plane_info[k]
                return (None, exts[b], d * BLKS, d, b)

            Es = {}
            VS = {}

            def emit_tail_d1(k):
                """Z-tree, recip, sm, v for plane k (E already computed)."""
                E = Es.pop(k)
                R = emit_tail_a(E)
                sm = smp.tile([128, BLKS, OUT_C], bf16, tag="sm", name="sm")
                nc.gpsimd.tensor_tensor(
                    sm[:], E[:],
                    R[:].unsqueeze(2).broadcast_to([128, BLKS, OUT_C]),
                    Alu.mult)
                v = vp.tile([128, BLKS, OUT_C], bf16, tag="v", name="v")
                nc.gpsimd.tensor_tensor(v[:], sm[:], subrep[:], Alu.subtract)
                VS[k] = v

            def emit_tail_d2(k):
                """channel max/min extremes for plane k."""
                v = VS.pop(k)
                _, ext, col, _, _ = pend_of(k)
                m1 = mmp.tile([128, BLKS, 16], bf16, tag="m1", name="m1")
                nc.vector.tensor_tensor(m1[:], v[:, :, 0:16], v[:, :, 16:32],
                                        Alu.max)
                m2 = mmp.tile([128, BLKS, 8], bf16, tag="m2", name="m2")
                nc.vector.tensor_tensor(m2[:], m1[:, :, 0:8], m1[:, :, 8:16],
                                        Alu.max)
                nc.vector.tensor_reduce(
                    ext[:, 0, col:col + BLKS], m2[:], axis=Ax.X, op=Alu.max)
                s1 = mmp.tile([128, BLKS, 16], f32, tag="s1", name="s1")
                nc.gpsimd.tensor_tensor(s1[:], v[:, :, 0:16], v[:, :, 16:32],
                                        Alu.add)
                n1 = mmp.tile([128, BLKS, 16], bf16, tag="n1", name="n1")
                nc.gpsimd.tensor_tensor(n1[:], s1[:], m1[:], Alu.subtract)
                n2 = mmp.tile([128, BLKS, 8], bf16, tag="n2", name="n2")
                nc.vector.tensor_tensor(n2[:], n1[:, :, 0:8], n1[:, :, 8:16],
                                        Alu.min)
                nc.vector.tensor_reduce(
                    ext[:, 1, col:col + BLKS], n2[:], axis=Ax.X, op=Alu.min)

            # Four-deep skewed pipeline: per iteration t, plane t runs steps
            # 0-1, t-1 runs 2-3, t-2 runs 4-5, t-3 runs 6-7+bias+exp (which
            # frees its psum cell); plane t-4 gets Z/recip/sm/v and t-5 its
            # chmax, each emitted in dependency-readiness order so no engine
            # queue head waits on same-iteration work.  4 cells live = all 8
            # psum banks; output transposes borrow a freed cell buffer.
            HALVED = {0, 1}
            emit_dma(0)
            emit_dma(1)
            for t in range(NP + 6):
                if 0 <= t - 5 < NP:
                    emit_tail_d2(t - 5)

                if 0 <= t - 3 < NP:
                    k = t - 3
                    emit_chain_step(slabs[k], cells[k], 6, k in HALVED)
                    emit_chain_step(slabs[k], cells[k], 7, k in HALVED)
                    emit_bias(cells[k])
                    Es[k] = emit_exp(cells[k])
                    del cells[k]
                    del slabs[k]
                if 0 <= t - 4 < NP:
                    emit_tail_d1(t - 4)
                if 0 <= t - 2 < NP:
                    emit_chain_step(slabs[t - 2], cells[t - 2], 4, (t - 2) in HALVED)
                    emit_chain_step(slabs[t - 2], cells[t - 2], 5, (t - 2) in HALVED)
                if 0 <= t - 1 < NP:
                    emit_chain_step(slabs[t - 1], cells[t - 1], 2, (t - 1) in HALVED)
                    emit_chain_step(slabs[t - 1], cells[t - 1], 3, (t - 1) in HALVED)
                if t < NP:
                    start_plane(t)
                    emit_dma(t + 2)
                    if not deferred[0]:
                        emit_deferred_consts()
                    emit_chain_step(slabs[t], cells[t], 0, t in HALVED)
                    emit_chain_step(slabs[t], cells[t], 1, t in HALVED)
            emit_b_final(0, exts[0])
            emit_b_final(1, exts[1])

    nc.compile()
    return nc


def _get_nc():
    if "nc" not in _COMPILED:
        _COMPILED["nc"] = build_kernel()
    return _COMPILED["nc"]


def build_in_maps(xs, wch, bias, subtract):
    brow = np.concatenate([
        np.ones(128, np.float32),
        np.tile(bias.astype(np.float32), BLKS),
    ]).reshape(1, 128 + BLKS * OUT_C).astype(ml_dtypes.bfloat16)
    subrep = np.tile(subtract[None, None, :], (128, BLKS, 1)).reshape(
        128, BLKS * OUT_C).astype(ml_dtypes.bfloat16)

    in_maps = []
    for c in range(N_CORES):
        in_maps.append({
            "xs": np.ascontiguousarray(xs[c * B_PER_CORE:(c + 1) * B_PER_CORE]),
            "wc": wch,
            "brow": brow,
            "subrep": subrep,
            "ident": np.eye(128, dtype=np.float32),
        })
    return in_maps


def kernel(x, weight, bias, subtract):
    from concourse.bass_utils import run_bass_kernel_spmd

    x = np.asarray(x, dtype=np.float32)
    weight = np.asarray(weight, dtype=np.float32)
    bias = np.asarray(bias, dtype=np.float32)
    subtract = np.asarray(subtract, dtype=np.float32)

    nc = _get_nc()

    xs = build_xstack(x)
    wch = build_wchain(weight).reshape(128, NSTEP * OUT_C).astype(W_NP_DT)
    in_maps = build_in_maps(xs, wch, bias, subtract)

    res = run_bass_kernel_spmd(nc, in_maps, core_ids=list(range(N_CORES)))
    outs = [res.results[c]["y"].reshape(B_PER_CORE, D, H, W)
            for c in range(N_CORES)]
    return np.concatenate(outs, axis=0)
